# revision 1
# baseline (speedup 1.0000x reference)
"""Trainium2 Bass kernel for nn_ExpertDistillationLoss.

Strategy (data-parallel over batch, 8 cores, 1 batch element each):
  - Device (per core): the FLOP-heavy expert-MSE pipeline.
      d.T[h, s] = W_s·sh.T − W_t·th.T  (bf16 operands, f32 PSUM accumulation,
      host-pre-transposed weight/activation layouts, W stationary)
      mean_base via d² accumulation + per-chunk ones-matmuls,
      cross+quad terms fused into one PSUM accumulator V[s, 256] built from
      (a) P-matmuls of d.T tiles against a host-prescaled B_cat and
      (b) Gram-matrix matmuls against host-precomputed G pairs,
      then one broadcasted DVE multiply/reduce against a_s/a_t.
      Device output per core: feat partial = Σ wsel·mse (1 scalar)
      (+ small debug tensors).
  - Host: input sharding/layout, the K=3 MC sampling scan (gates-only, exact
    argmax semantics), method-B losses, and the final scalar combine.
"""

import numpy as np
import ml_dtypes

B, S, H, E, R, K = 8, 2048, 2048, 8, 16, 3
ALPHA = 0.5
LAMBDA_COV = 0.5
BETA_ENT = 0.1
TEMP_LO, TEMP_HI = 0.5, 1.5
SCALE_T = 2.0
SCALE_S = 2.0
EPS = 1e-8

NK = H // 128          # 16 k-tiles
NM = H // 128          # 16 output h-tiles
NNS = 4                # s-chunks of 512
NSUB = 4               # 128-token subchunks per s-chunk
NCHUNK = S // 128      # 16

BF16 = ml_dtypes.bfloat16
FP8 = ml_dtypes.float8_e4m3fn
WSCALE = 64.0   # weights pre-scaled by this so fp8 e4m3 stays in normal range

_PROGRAM_CACHE = {}


# ----------------------------------------------------------------------------
# device program
# ----------------------------------------------------------------------------

def _build_program(db_nonzero: bool, debug_out: bool = False):
    import concourse.bacc as bacc
    import concourse.tile as tile
    from concourse import mybir

    f32 = mybir.dt.float32
    bf16 = mybir.dt.bfloat16
    fp8 = mybir.dt.float8e4
    DR = mybir.MatmulPerfMode.DoubleRow
    ALU = mybir.AluOpType
    AX = mybir.AxisListType

    kt = NK + (1 if db_nonzero else 0)   # extra k-tile carries the bias row

    nc = bacc.Bacc("TRN2", target_bir_lowering=False, debug=False)

    # DRAM inputs (per-core shapes; layouts are host-prepared)
    d_shT = nc.dram_tensor("shT", [128, kt, S], fp8, kind="ExternalInput").ap()
    d_thT = nc.dram_tensor("thT", [128, NK, S], fp8, kind="ExternalInput").ap()
    d_Ws = nc.dram_tensor("Ws", [NM, 128, kt, 128], fp8, kind="ExternalInput").ap()
    d_Wt = nc.dram_tensor("Wt", [NM, 128, NK, 128], fp8, kind="ExternalInput").ap()
    d_Bc = nc.dram_tensor("Bcat", [128, NM, 256], bf16, kind="ExternalInput").ap()
    d_Gs = nc.dram_tensor("Gs", [16, 256], bf16, kind="ExternalInput").ap()
    d_Gt = nc.dram_tensor("Gt", [16, 256], bf16, kind="ExternalInput").ap()
    d_acat = nc.dram_tensor("acat", [128, NCHUNK, 32], f32, kind="ExternalInput").ap()
    d_asT = nc.dram_tensor("asT", [16, S], bf16, kind="ExternalInput").ap()
    d_atT = nc.dram_tensor("atT", [16, S], bf16, kind="ExternalInput").ap()
    d_wsel = nc.dram_tensor("wsel", [128, 128], f32, kind="ExternalInput").ap()
    d_wsele = nc.dram_tensor("wsel_e", [128, 16], f32, kind="ExternalInput").ap()
    d_onesH = nc.dram_tensor("onesH", [128, 1], f32, kind="ExternalInput").ap()
    d_ones1 = nc.dram_tensor("ones1", [128, 1], f32, kind="ExternalInput").ap()

    # outputs
    d_feat = nc.dram_tensor("feat", [1, 1], f32, kind="ExternalOutput").ap()
    if debug_out:
        d_msed = nc.dram_tensor("mse_dbg", [128, 128], f32, kind="ExternalOutput").ap()
        d_mbd = nc.dram_tensor("mb_dbg", [128, 16], f32, kind="ExternalOutput").ap()
        d_dtd = nc.dram_tensor("dt_dbg", [NM, 128, 512], bf16, kind="ExternalOutput").ap()
        d_accd = nc.dram_tensor("acc_dbg", [128, S], f32, kind="ExternalOutput").ap()

    with tile.TileContext(nc) as tc:
        with (
            tc.tile_pool(name="const", bufs=1) as cp,
            tc.tile_pool(name="wst", bufs=6) as wp,
            tc.tile_pool(name="dT", bufs=2) as dp,
            tc.tile_pool(name="sq", bufs=2) as qp,
            tc.tile_pool(name="vc", bufs=2) as vp,
        ):
            from contextlib import ExitStack
            _mp = ExitStack()
            pd = _mp.enter_context(tc.tile_pool(name="pd", bufs=3, space="PSUM"))
            pv = _mp.enter_context(tc.tile_pool(name="pv", bufs=5, space="PSUM"))
            # ---- resident loads ----
            # DMA emission order matters for startup: the first m-tiles' W
            # stripes and the first s-chunk's activation slices go first so
            # PE can start ~15us in instead of waiting for the bulk load.
            NPRE = 3
            whead = []
            for m in range(NPRE):
                ws0 = wp.tile([128, kt * 128], fp8, tag="w", name=f"wsh_{m}")
                nc.sync.dma_start(ws0[:], d_Ws[m].rearrange("p a b -> p (a b)"))
                wt0 = wp.tile([128, NK * 128], fp8, tag="w", name=f"wth_{m}")
                nc.sync.dma_start(wt0[:], d_Wt[m].rearrange("p a b -> p (a b)"))
                whead.append((ws0, wt0))

            shT = cp.tile([128, kt * S], fp8, tag="shT")
            thT = cp.tile([128, NK * S], fp8, tag="thT")
            for c0, c1 in ((0, 1024), (1024, S)):
                for k in range(kt):
                    nc.sync.dma_start(shT[:, k * S + c0:k * S + c1],
                                      d_shT[:, k, c0:c1])
                    if k < NK:
                        nc.sync.dma_start(thT[:, k * S + c0:k * S + c1],
                                          d_thT[:, k, c0:c1])
            Bc = cp.tile([128, NM * 256], bf16, tag="Bc")
            nc.sync.dma_start(Bc[:], d_Bc[:].rearrange("p a b -> p (a b)"))
            Gs = cp.tile([16, 256], bf16, tag="Gs")
            nc.sync.dma_start(Gs[:], d_Gs)
            Gt = cp.tile([16, 256], bf16, tag="Gt")
            nc.sync.dma_start(Gt[:], d_Gt)
            acat_sb = cp.tile([128, NCHUNK * 32], f32, tag="acat")
            nc.sync.dma_start(acat_sb[:], d_acat[:].rearrange("p a b -> p (a b)"))
            asT_sb = cp.tile([16, S], bf16, tag="asT")
            nc.sync.dma_start(asT_sb[:], d_asT)
            atT_sb = cp.tile([16, S], bf16, tag="atT")
            nc.sync.dma_start(atT_sb[:], d_atT)
            wsel = cp.tile([128, 128], f32, tag="wsel")
            nc.sync.dma_start(wsel[:], d_wsel)
            wsele = cp.tile([128, 16], f32, tag="wsele")
            nc.sync.dma_start(wsele[:], d_wsele)
            onesH = cp.tile([128, 1], f32, tag="onesH")
            nc.sync.dma_start(onesH[:], d_onesH)
            ones1 = cp.tile([128, 1], f32, tag="ones1")
            nc.sync.dma_start(ones1[:], d_ones1)

            acc128 = cp.tile([128, S], f32, tag="acc128")
            nc.vector.memset(acc128[:], 0.0)
            mse_sb = cp.tile([128, 128], f32, tag="mse")
            mb_sb = cp.tile([128, 16], f32, tag="mb")

            # ---- main loop: s-chunk pairs sharing one W load ----
            # dTc caches the second chunk's d tiles so its P-matmuls (and the
            # 4-bank V accumulation) run after the first chunk's V is consumed.
            dTc = cp.tile([128, NM * 512], bf16, tag="dTc")

            def consume_v(Vt, base_chunk):
                for sub in range(NSUB):
                    chunk = base_chunk + sub
                    ab = acat_sb[:, chunk * 32:(chunk + 1) * 32].rearrange(
                        "p (t r) -> p t r", t=2)
                    ab = ab.unsqueeze(2).broadcast_to([128, 2, 8, 16])
                    prod = vp.tile([128, 256], f32, tag="prod",
                                   name=f"prod_{chunk}")
                    nc.vector.tensor_tensor(
                        prod[:].rearrange("p (t e r) -> p t e r", t=2, e=8),
                        Vt[sub][:].rearrange("p (t e r) -> p t e r", t=2, e=8),
                        ab, ALU.mult)
                    red = vp.tile([128, 16], f32, tag="red", name=f"red_{chunk}")
                    nc.vector.tensor_reduce(
                        red[:], prod[:].rearrange("p (t e r) -> p t e r", t=2, e=8),
                        axis=AX.X, op=ALU.add)
                    nc.vector.tensor_add(mse_sb[:, chunk * 8:(chunk + 1) * 8],
                                         red[:, 0:8], red[:, 8:16])

            def u_mms(Vt, s0):
                for sub in range(NSUB):
                    t0 = s0 + sub * 128
                    nc.tensor.matmul(Vt[sub][:], asT_sb[:, t0:t0 + 128],
                                     Gs[:], start=True, stop=False)
                    nc.tensor.matmul(Vt[sub][:], atT_sb[:, t0:t0 + 128],
                                     Gt[:], start=False, stop=False)

            for nsp in range(NNS // 2):
                s0a = nsp * 1024
                s0b = s0a + 512
                Va = [pv.tile([128, 256], f32, tag="V", name=f"Va_{nsp}_{j}")
                      for j in range(NSUB)]
                for m in range(NM):
                    if nsp == 0 and m < NPRE:
                        ws, wt = whead[m]
                    else:
                        ws = wp.tile([128, kt * 128], fp8, tag="w",
                                     name=f"ws_{nsp}_{m}")
                        wsf = d_Ws[m].rearrange("p a b -> p (a b)")
                        hw_ = (kt * 128) // 2
                        nc.sync.dma_start(ws[:, 0:hw_], wsf[:, 0:hw_])
                        nc.sync.dma_start(ws[:, hw_:kt * 128], wsf[:, hw_:kt * 128])
                        wt = wp.tile([128, NK * 128], fp8, tag="w",
                                     name=f"wt_{nsp}_{m}")
                        wtf = d_Wt[m].rearrange("p a b -> p (a b)")
                        nc.sync.dma_start(wt[:, 0:NK * 64], wtf[:, 0:NK * 64])
                        nc.sync.dma_start(wt[:, NK * 64:NK * 128], wtf[:, NK * 64:NK * 128])

                    ws3 = ws[:].rearrange("p (k c) -> p k c", k=kt)
                    wt3 = wt[:].rearrange("p (k c) -> p k c", k=NK)
                    sh3 = shT[:].rearrange("p (k s) -> p k s", k=kt)
                    th3 = thT[:].rearrange("p (k s) -> p k s", k=NK)
                    NPAIR = NK // 2
                    pds = []
                    for half, s0 in ((0, s0a), (1, s0b)):
                        pd_t = pd.tile([128, 512], f32, tag="pd",
                                       name=f"pd_{nsp}_{m}_{half}")
                        pds.append(pd_t)
                        for kp in range(NPAIR):
                            nc.tensor.matmul(
                                pd_t[:], ws3[:, 2 * kp:2 * kp + 2, :],
                                sh3[:, 2 * kp:2 * kp + 2, s0:s0 + 512],
                                start=(kp == 0), stop=False, perf_mode=DR)
                            nc.tensor.matmul(
                                pd_t[:], wt3[:, 2 * kp:2 * kp + 2, :],
                                th3[:, 2 * kp:2 * kp + 2, s0:s0 + 512],
                                start=False,
                                stop=(kp == NPAIR - 1 and kt == NK),
                                perf_mode=DR)
                        if kt > NK:
                            # bias tail tile: plain (non-DoubleRow) fp8 matmul
                            nc.tensor.matmul(
                                pd_t[:], ws3[:, NK:NK + 1, :],
                                sh3[:, NK:NK + 1, s0:s0 + 512],
                                start=False, stop=True)
                        if half == 0:
                            # dT copy runs on ACT while PE streams half1's
                            # k-loop, so the P-matmuls below don't stall PE
                            dT = dp.tile([128, 512], bf16, tag="dT",
                                         name=f"dT_{nsp}_{m}")
                            nc.scalar.copy(dT[:], pds[0][:])
                            if debug_out and nsp == 0:
                                nc.sync.dma_start(d_dtd[m], dT[:])
                            if m == 0:
                                u_mms(Va, s0a)

                    for half, s0 in ((0, s0a), (1, s0b)):
                        pd_t = pds[half]
                        sq = qp.tile([128, 512], f32, tag="sq",
                                     name=f"sq_{nsp}_{m}_{half}")
                        nc.scalar.square(sq[:], pd_t[:])
                        nc.vector.tensor_add(acc128[:, s0:s0 + 512],
                                             acc128[:, s0:s0 + 512], sq[:])
                        if half == 0:
                            for sub in range(NSUB):
                                nc.tensor.matmul(Va[sub][:],
                                                 dT[:, sub * 128:(sub + 1) * 128],
                                                 Bc[:, m * 256:(m + 1) * 256],
                                                 start=False, stop=(m == NM - 1))
                        else:
                            nc.scalar.copy(dTc[:, m * 512:(m + 1) * 512], pd_t[:])

                consume_v(Va, nsp * NSUB * 2)

                Vb = [pv.tile([128, 256], f32, tag="V", name=f"Vb_{nsp}_{j}")
                      for j in range(NSUB)]
                u_mms(Vb, s0b)
                for m in range(NM):
                    for sub in range(NSUB):
                        nc.tensor.matmul(Vb[sub][:],
                                         dTc[:, m * 512 + sub * 128: m * 512 + (sub + 1) * 128],
                                         Bc[:, m * 256:(m + 1) * 256],
                                         start=False, stop=(m == NM - 1))
                consume_v(Vb, nsp * NSUB * 2 + NSUB)

            # ---- mean_base: per-chunk ones-matmuls ----
            _mp.close()
            pm_ctx = tc.tile_pool(name="pm", bufs=1, space="PSUM")
            pm = pm_ctx.__enter__()
            mbp = pm.tile([128, 512], f32, tag="pmisc")
            for c in range(NCHUNK):
                nc.tensor.matmul(mbp[:, c:c + 1], acc128[:, c * 128:(c + 1) * 128],
                                 onesH[:], start=True, stop=True)
            nc.scalar.copy(mb_sb[:], mbp[:, 0:16])

            # ---- feat partial ----
            scr1 = cp.tile([128, 128], f32, tag="scr1")
            fx = cp.tile([128, 1], f32, tag="fx")
            nc.vector.tensor_mul(scr1[:], mse_sb[:], wsel[:])
            nc.vector.tensor_reduce(fx[:], scr1[:], axis=AX.X, op=ALU.add)
            scr2 = cp.tile([128, 16], f32, tag="scr2")
            fmb = cp.tile([128, 1], f32, tag="fmb")
            nc.vector.tensor_mul(scr2[:], mb_sb[:], wsele[:])
            nc.vector.tensor_reduce(fmb[:], scr2[:], axis=AX.X, op=ALU.add)
            fsum = cp.tile([128, 1], f32, tag="fsum")
            nc.vector.tensor_add(fsum[:], fx[:], fmb[:])
            fp = pm.tile([128, 512], f32, tag="pmisc")
            nc.tensor.matmul(fp[0:1, 0:1], fsum[:], ones1[:], start=True, stop=True)
            fout = cp.tile([1, 1], f32, tag="fout")
            nc.scalar.copy(fout[:], fp[0:1, 0:1])

            pm_ctx.__exit__(None, None, None)
            nc.sync.dma_start(d_feat, fout[:])
            if debug_out:
                nc.sync.dma_start(d_msed, mse_sb[:])
                nc.sync.dma_start(d_mbd, mb_sb[:])
                nc.sync.dma_start(d_accd, acc128[:])

    nc.compile()
    return nc


def _get_program(db_nonzero: bool, debug_out: bool = False):
    key = (bool(db_nonzero), bool(debug_out))
    if key not in _PROGRAM_CACHE:
        _PROGRAM_CACHE[key] = _build_program(*key)
    return _PROGRAM_CACHE[key]


# ----------------------------------------------------------------------------
# host side
# ----------------------------------------------------------------------------

def _host_scan_all(tg_all, sg_all, mask_f, gumbel):
    """Method-A sampling scan, all cores vectorized. Exact argmax semantics.
    Returns (wsel[B,S,E] f32, wsum f64, t_counts[E] f64, s_counts[E] f64)."""
    f32 = np.float32
    p = tg_all.astype(f32).copy()
    wsel = np.zeros((B, S, E), f32)
    BIG = f32(1e4)
    iota = np.arange(E, dtype=f32)
    for k in range(K):
        z = np.log(p) + gumbel[k]
        m = z.max(-1, keepdims=True)
        ge = (z >= m).astype(f32)
        t = iota + BIG - BIG * ge
        idxf = t.min(-1, keepdims=True)
        oh = (iota == idxf).astype(f32)
        po = p * oh
        w = po.sum(-1)
        sg_k = (sg_all * oh).sum(-1)
        mw = mask_f * w
        wsel += mw[..., None] * oh
        if k < K - 1:
            pn = p + (ALPHA - 1.0) * po
            p = pn / pn.sum(-1, keepdims=True)
    # counts from wsel (mw·oh summed over k) and the student-gate variant
    t_counts = wsel.astype(np.float64).sum(axis=(0, 1))
    wsum = float(t_counts.sum())
    # recompute s-side accumulation (needs per-step oh); cheap second pass
    p = tg_all.astype(f32).copy()
    s_counts = np.zeros(E, np.float64)
    for k in range(K):
        z = np.log(p) + gumbel[k]
        m = z.max(-1, keepdims=True)
        ge = (z >= m).astype(f32)
        t = iota + BIG - BIG * ge
        idxf = t.min(-1, keepdims=True)
        oh = (iota == idxf).astype(f32)
        po = p * oh
        sg_k = (sg_all * oh).sum(-1)
        s_counts += ((mask_f * sg_k)[..., None] * oh).astype(np.float64).sum(axis=(0, 1))
        if k < K - 1:
            pn = p + (ALPHA - 1.0) * po
            p = pn / pn.sum(-1, keepdims=True)
    return wsel, wsum, t_counts, s_counts


def _host_method_b(tg, sg, temp_c):
    """Per-core method-B partials: (tkl, ent)."""
    f32 = np.float32
    tg = tg.astype(f32)
    sg = sg.astype(f32)
    sgT = sg / f32(temp_c)
    ltg = np.log(tg)
    lsg = np.log(sg)
    ent = (sg * lsg).sum(dtype=f32)
    mb2 = sgT.max(-1, keepdims=True)
    ex = np.exp(sgT - mb2)
    se = ex.sum(-1, keepdims=True, dtype=f32)
    lse = np.log(se) + mb2
    sum_tg = tg.sum(-1, keepdims=True, dtype=f32)
    tkl = (tg * (ltg - sgT)).sum(dtype=f32) + (lse * sum_tg).sum(dtype=f32)
    return tkl, ent


def _prep_shared(inputs, db_nonzero):
    """Replicated (per-core identical) device arrays."""
    f32 = np.float32
    W_t = np.asarray(inputs["W_t"], f32)
    W_s = np.asarray(inputs["W_s"], f32)
    A_t = np.asarray(inputs["A_t"], f32)
    A_s = np.asarray(inputs["A_s"], f32)
    B_t = np.asarray(inputs["B_t"], f32)
    B_s = np.asarray(inputs["B_s"], f32)
    db = (np.asarray(inputs["b_s"], f32) - np.asarray(inputs["b_t"], f32))

    kt = NK + (1 if db_nonzero else 0)

    # W layout [m, p, k, c] = W[m*128+c, k*128+p]; fp8 values pre-scaled by
    # WSCALE so the e4m3 normal range covers N(0, 1/H) weights.
    def w_host(W, k_tiles, bias=None):
        out = np.zeros((NM, 128, k_tiles, 128), FP8)
        out[:, :, :NK, :] = (
            (W * WSCALE).astype(FP8).reshape(NM, 128, NK, 128).transpose(0, 3, 2, 1)
        )
        if bias is not None and k_tiles > NK:
            # bias block: partition 0 row carries db[m*128+c]
            out[:, 0, NK, :] = (bias * WSCALE).astype(FP8).reshape(NM, 128)
        return np.ascontiguousarray(out)

    Ws = w_host(W_s, kt, db if db_nonzero else None)
    Wt = w_host(-W_t, NK)   # negated: PSUM accumulation adds, d = base_s - base_t


    # Bcat [p, m, 256]; /WSCALE compensates dT carrying WSCALE*d
    Bs_her = B_s.transpose(1, 0, 2).reshape(H, E * R)
    Bt_her = B_t.transpose(1, 0, 2).reshape(H, E * R)
    B_cat = np.concatenate(
        [(2.0 * SCALE_S / (H * WSCALE)) * Bs_her,
         (-2.0 * SCALE_T / (H * WSCALE)) * Bt_her], axis=1
    ).astype(BF16)
    Bcat = np.ascontiguousarray(B_cat.reshape(NM, 128, 256).transpose(1, 0, 2))

    # Gram pairs [16, 256]
    G_ss = np.einsum("ehr,ehq->erq", B_s, B_s)
    G_st = np.einsum("ehr,ehq->erq", B_s, B_t)
    G_tt = np.einsum("ehr,ehq->erq", B_t, B_t)
    G_stT = G_st.transpose(0, 2, 1)

    def to_req(G):
        return G.transpose(1, 0, 2).reshape(R, E * R)

    Gs = np.concatenate(
        [(SCALE_S * SCALE_S / H) * to_req(G_ss),
         (-SCALE_S * SCALE_T / H) * to_req(G_st)], axis=1).astype(BF16)
    Gt = np.concatenate(
        [(-SCALE_S * SCALE_T / H) * to_req(G_stT),
         (SCALE_T * SCALE_T / H) * to_req(G_tt)], axis=1).astype(BF16)

    onesH = np.full((128, 1), 1.0 / (H * WSCALE * WSCALE), f32)
    ones1 = np.ones((128, 1), f32)

    shared = dict(Ws=Ws, Wt=Wt, Bcat=Bcat, Gs=Gs, Gt=Gt,
                  onesH=onesH, ones1=ones1)
    mats = dict(A_sT=np.ascontiguousarray(A_s.T), A_tT=np.ascontiguousarray(A_t.T))
    return shared, mats, kt


def _prep_core(inputs, core, kt, wsel, mats):
    """Per-core device arrays."""
    f32 = np.float32
    sh = np.asarray(inputs["student_hidden_states"][core], f32)
    th = np.asarray(inputs["teacher_hidden_states"][core], f32)

    a_s = sh @ mats["A_sT"]                      # [S, R] f32
    a_t = th @ mats["A_tT"]
    acat = np.concatenate([a_s, a_t], axis=1)    # [S, 32]
    acat = np.ascontiguousarray(
        acat.reshape(NCHUNK, 128, 32).transpose(1, 0, 2)).astype(f32)
    asT = np.ascontiguousarray(a_s.T).astype(BF16)
    atT = np.ascontiguousarray(a_t.T).astype(BF16)

    # [p, k, s] layout of x.T (k = inner dim of x)
    def xt_host(x, k_tiles, ones_tail=False):
        out = np.zeros((128, k_tiles, S), FP8)
        out[:, :NK, :] = x.T.astype(FP8).reshape(NK, 128, S).transpose(1, 0, 2)
        if ones_tail and k_tiles > NK:
            out[0, NK, :] = FP8(1.0)
        return np.ascontiguousarray(out)

    shT = xt_host(sh, kt, ones_tail=(kt > NK))
    thT = xt_host(th, NK)

    wsel_dev = np.ascontiguousarray(
        wsel.reshape(NCHUNK, 128, E).transpose(1, 0, 2).reshape(128, 128)).astype(f32)
    wsel_e = np.ascontiguousarray(wsel.sum(-1).reshape(NCHUNK, 128).T).astype(f32)
    return dict(shT=shT, thT=thT, wsel=wsel_dev, wsel_e=wsel_e,
                acat=acat, asT=asT, atT=atT)


def _combine(feat_parts, wsum, t_counts, s_counts, tkls, ents, temp_c):
    f32 = np.float32
    feat = np.sum(np.asarray(feat_parts, f32), dtype=f32)
    tc = np.asarray(t_counts, np.float64)
    sc = np.asarray(s_counts, np.float64)
    tkl = np.sum(np.asarray(tkls, f32), dtype=f32)
    ent = np.sum(np.asarray(ents, f32), dtype=f32)

    feat_loss = feat / max(wsum, 1e-8)
    t_avg = tc / tc.sum() + EPS
    s_avg = sc / sc.sum() + EPS
    t_avg = t_avg / t_avg.sum()
    s_avg = s_avg / s_avg.sum()
    coverage_kl = (t_avg * (np.log(t_avg) - np.log(s_avg))).sum() / E
    method_a_total = feat_loss + LAMBDA_COV * coverage_kl
    temp_kl = tkl / B
    entropy_loss = ent / (B * S)
    method_b_total = temp_kl + BETA_ENT * entropy_loss
    return np.array(
        [feat_loss, coverage_kl, method_a_total, temp_kl, entropy_loss,
         method_b_total, temp_c], f32)


def _host_all(inputs):
    """Host scan/method-B for all cores + per-core device input maps."""
    f32 = np.float32
    db_nonzero = bool(
        np.any(np.asarray(inputs["b_s"], f32) != np.asarray(inputs["b_t"], f32)))
    temp = float(np.asarray(inputs["temperature"], f32))
    temp_c = float(np.clip(temp, TEMP_LO, TEMP_HI))

    u = np.asarray(inputs["uniform_noise"], f32)
    gumbel = -np.log(-np.log(u * (1.0 - 2e-7) + 1e-7)).astype(f32)
    mask_f = np.asarray(inputs["attention_mask"], f32)
    tg_all = np.asarray(inputs["teacher_gates"], f32)
    sg_all = np.asarray(inputs["student_gates"], f32)

    shared, mats, kt = _prep_shared(inputs, db_nonzero)
    wsel_all, wsum, t_counts, s_counts = _host_scan_all(
        tg_all, sg_all, mask_f, gumbel)

    in_maps = []
    tkls, ents = [], []
    for c in range(B):
        tkl, ent = _host_method_b(tg_all[c], sg_all[c], temp_c)
        tkls.append(tkl)
        ents.append(ent)
        m = dict(shared)
        m.update(_prep_core(inputs, c, kt, wsel_all[c], mats))
        in_maps.append(m)

    return dict(in_maps=in_maps, db_nonzero=db_nonzero, temp_c=temp_c,
                wsum=wsum, t_counts=t_counts, s_counts=s_counts,
                tkls=tkls, ents=ents)


def kernel(**inputs) -> np.ndarray:
    host = _host_all(inputs)
    nc = _get_program(host["db_nonzero"])

    from concourse.bass_utils import run_bass_kernel_spmd

    res = run_bass_kernel_spmd(nc, host["in_maps"], core_ids=list(range(B)))
    feat_parts = [float(res.results[c]["feat"][0, 0]) for c in range(B)]

    return _combine(feat_parts, host["wsum"], host["t_counts"],
                    host["s_counts"], host["tkls"], host["ents"],
                    host["temp_c"])



# revision 11
# speedup vs baseline: 1.2891x; 1.2891x over previous
"""Trainium2 Bass kernel for nn_ExpertDistillationLoss.

Strategy (data-parallel over batch, 8 cores, 1 batch element each):
  - Device (per core): the FLOP-heavy expert-MSE pipeline.
      d.T[h, s] = W_s.sh.T - W_t.th.T computed as one concatenated fp8
      DoubleRow GEMM (W stationary & SBUF-resident, loaded once; host
      pre-transposed layouts; f32 PSUM accumulation).
      mean_base via ACT square + per-tile ones-matmul PSUM accumulation.
      cross+quad terms fused into one PSUM accumulator V[s, 256] built from
      (a) fp8 DoubleRow P-matmuls of dT m-tile PAIRS against host-prescaled
          B_cat and
      (b) one fp8 DoubleRow Gram matmul per token tile (as/at paired),
      then a broadcasted DVE multiply/reduce against a_s/a_t.
      Device output per core: feat partial = sum wsel*mse (1 scalar).
  - Host: input sharding/layout, the K=3 MC sampling scan (gates-only, exact
    argmax semantics), method-B losses, and the final scalar combine.
"""

import numpy as np
import ml_dtypes

B, S, H, E, R, K = 8, 2048, 2048, 8, 16, 3
ALPHA = 0.5
LAMBDA_COV = 0.5
BETA_ENT = 0.1
TEMP_LO, TEMP_HI = 0.5, 1.5
SCALE_T = 2.0
SCALE_S = 2.0
EPS = 1e-8

NM = 16                # output h-tiles (128 rows each)
NKX = 32               # k-tiles: 16 student + 16 teacher
NC4 = 4                # 512-token chunks
NSUB = 4               # 128-token subchunks per chunk
NCH = 16               # 128-token chunks over S

BF16 = ml_dtypes.bfloat16
FP8 = ml_dtypes.float8_e4m3fn
WSCALE = 64.0          # W pre-scale so fp8 e4m3 stays in normal range
DCOPY = 0.25           # dT = DCOPY * pd = (WSCALE*DCOPY) * d = 16 d
ALPHA_V = 131072.0     # 2**17: common scale carried by the V accumulator
BC_F = ALPHA_V * 2.0 * SCALE_S / (H * WSCALE * DCOPY)   # = 16.0
GC_F = ALPHA_V * SCALE_S * SCALE_T / H                  # = 256.0

_PROGRAM_CACHE = {}


# ----------------------------------------------------------------------------
# device program
# ----------------------------------------------------------------------------

def _build_program(db_nonzero: bool, debug_out: bool = False):
    import concourse.bacc as bacc
    import concourse.tile as tile
    from concourse import mybir

    f32 = mybir.dt.float32
    fp8 = mybir.dt.float8e4
    DR = mybir.MatmulPerfMode.DoubleRow
    ALU = mybir.AluOpType
    AX = mybir.AxisListType

    KT = NKX + (1 if db_nonzero else 0)   # extra k-tile carries the bias row
    WB = KT * 128                          # W cols per m-tile
    XB = KT * 512                          # x cols per 512-token chunk

    nc = bacc.Bacc("TRN2", target_bir_lowering=False, debug=False)

    # DRAM inputs (per-core shapes; layouts are host-prepared)
    d_xc = nc.dram_tensor("xc", [128, NC4, XB], fp8, kind="ExternalInput").ap()
    d_Wc = nc.dram_tensor("Wc", [128, NM, WB], fp8, kind="ExternalInput").ap()
    d_Bc = nc.dram_tensor("Bc", [128, 8 * 512], fp8, kind="ExternalInput").ap()
    d_Gc = nc.dram_tensor("Gc", [16, 512], fp8, kind="ExternalInput").ap()
    d_aT = nc.dram_tensor("aT", [16, 2 * S], fp8, kind="ExternalInput").ap()
    d_ac = nc.dram_tensor("acat", [128, NCH * 32], f32, kind="ExternalInput").ap()
    d_wsel = nc.dram_tensor("wsel", [128, 128], f32, kind="ExternalInput").ap()
    d_wsele = nc.dram_tensor("wsel_e", [128, 16], f32, kind="ExternalInput").ap()
    d_onesH = nc.dram_tensor("onesH", [128, 1], f32, kind="ExternalInput").ap()
    d_ones1 = nc.dram_tensor("ones1", [128, 1], f32, kind="ExternalInput").ap()

    d_feat = nc.dram_tensor("feat", [1, 1], f32, kind="ExternalOutput").ap()
    if debug_out:
        d_msed = nc.dram_tensor("mse_dbg", [128, 128], f32, kind="ExternalOutput").ap()
        d_mbd = nc.dram_tensor("mb_dbg", [128, 16], f32, kind="ExternalOutput").ap()

    with tile.TileContext(nc) as tc:
        with (
            tc.tile_pool(name="const", bufs=1) as cp,
            tc.tile_pool(name="dT", bufs=2) as dp,
            tc.tile_pool(name="sq", bufs=3) as qp,
            tc.tile_pool(name="vc", bufs=4) as vp,
            tc.tile_pool(name="pd", bufs=2, space="PSUM") as pd,
            tc.tile_pool(name="pv", bufs=4, space="PSUM") as pv,
            tc.tile_pool(name="pm", bufs=2, space="PSUM") as pm,
        ):
            # ---- SBUF tiles ----
            Gc_sb = cp.tile([16, 512], fp8, tag="Gc")
            aT_sb = cp.tile([16, 2 * S], fp8, tag="aT")
            Wc = cp.tile([128, NM * WB], fp8, tag="Wc")
            xc = cp.tile([128, NC4 * XB], fp8, tag="xc")
            Bc = cp.tile([128, 8 * 512], fp8, tag="Bc")
            acat_sb = cp.tile([128, NCH * 32], f32, tag="acat")
            wsel = cp.tile([128, 128], f32, tag="wsel")
            wsele = cp.tile([128, 16], f32, tag="wsele")
            onesH = cp.tile([128, 1], f32, tag="onesH")
            ones1 = cp.tile([128, 1], f32, tag="ones1")
            mse_sb = cp.tile([128, 128], f32, tag="mse")
            mb_sb = cp.tile([128, 16], f32, tag="mb")

            # ---- DMA emission order (HWDGE serializes at ~625ns/DMA and the
            # DMA bus at ~360B/ns; order = need order on the PE) ----
            dma = nc.sync.dma_start
            dma(Gc_sb[:], d_Gc)
            dma(aT_sb[:], d_aT)
            dma(Wc[:, 0:256], d_Wc[:, 0, 0:256])          # m0 kp0
            dma(xc[:, 0:2048], d_xc[:, 0, 0:2048])        # c0 k0-3
            dma(Wc[:, 256:WB], d_Wc[:, 0, 256:WB])        # m0 rest
            dma(xc[:, 2048:8192], d_xc[:, 0, 2048:8192])  # c0 k4-15
            dma(xc[:, 8192:XB], d_xc[:, 0, 8192:XB])      # c0 k16-31(+bias)
            dma(Wc[:, WB:2 * WB], d_Wc[:, 1, :])
            dma(Bc[:], d_Bc)
            for m in (2, 3, 4):
                dma(Wc[:, m * WB:(m + 1) * WB], d_Wc[:, m, :])
            dma(acat_sb[:], d_ac)
            dma(wsel[:], d_wsel)
            dma(wsele[:], d_wsele)
            dma(onesH[:], d_onesH)
            dma(ones1[:], d_ones1)
            for m in (5, 6, 7, 8):
                dma(Wc[:, m * WB:(m + 1) * WB], d_Wc[:, m, :])
            dma(xc[:, XB:2 * XB], d_xc[:, 1, :])
            for m in (9, 10, 11, 12):
                dma(Wc[:, m * WB:(m + 1) * WB], d_Wc[:, m, :])
            dma(xc[:, 2 * XB:3 * XB], d_xc[:, 2, :])
            for m in (13, 14, 15):
                dma(Wc[:, m * WB:(m + 1) * WB], d_Wc[:, m, :])
            dma(xc[:, 3 * XB:4 * XB], d_xc[:, 3, :])

            # ---- views ----
            W4 = Wc[:].rearrange("p (m k c) -> p m k c", m=NM, k=KT)
            x4 = xc[:].rearrange("p (n k s) -> p n k s", n=NC4, k=KT)
            aT2 = aT_sb[:].rearrange("p (j s) -> p j s", j=2)
            Gc2 = Gc_sb[:].rearrange("p (j g) -> p j g", j=2)
            Bc3 = Bc[:].rearrange("p (mp j g) -> p mp j g", mp=8, j=2)

            # PSUM rule (probe-verified): a bank holds ONE open accumulation
            # group; a start=True wipes any other OPEN group's partials in
            # that bank (committed/stopped results survive). So: V banks run
            # one sub's full chain at a time (subs 0/2 during the m-loop,
            # subs 1/3 afterwards from the dT cache), and mean_base uses
            # per-(m,sub) single-shot matmuls + a DVE reduction over m.
            V_of = {}    # c -> [2 psum tiles of [128, 512] (2 subs each)]
            mb_of = {}   # c -> [128, 64] psum tile of per-(m,sub) sums
            sq_of = {}   # (c, m) -> sq tile
            dT_of = {}   # c -> [128, 8*1024] fp8 dT cache (mp, j, 512)

            def Vap(c, sub):
                t = V_of[c][sub // 2]
                return t[:, (sub % 2) * 256:(sub % 2) * 256 + 256]

            def emit_u(c, subs):
                for sub in subs:
                    t0 = c * 512 + sub * 128
                    nc.tensor.matmul(Vap(c, sub), aT2[:, :, t0:t0 + 128],
                                     Gc2, start=True, stop=False,
                                     perf_mode=DR)

            def emit_pmm(c, mp, subs):
                dT3 = dT_of[c][:].rearrange("p (mp j s) -> p mp j s",
                                            mp=8, j=2)
                for sub in subs:
                    nc.tensor.matmul(
                        Vap(c, sub),
                        dT3[:, mp, :, sub * 128:sub * 128 + 128],
                        Bc3[:, mp], start=False, stop=(mp == 7),
                        perf_mode=DR)

            def emit_start(c):
                V_of[c] = [pv.tile([128, 512], f32, tag="V", name=f"V_{c}_{i}")
                           for i in range(2)]
                mb_of[c] = pm.tile([128, 64], f32, tag="mbp", name=f"mb_{c}")
                dT_of[c] = dp.tile([128, 8 * 1024], fp8, tag="dT",
                                   name=f"dTall_{c}")
                emit_u(c, (0, 2))

            def emit_kloop(c, m):
                pdt = pd.tile([128, 512], f32, tag="pd", name=f"pd_{c}_{m}")
                for kp in range(NKX // 2):
                    nc.tensor.matmul(
                        pdt[:], W4[:, m, 2 * kp:2 * kp + 2, :],
                        x4[:, c, 2 * kp:2 * kp + 2, :],
                        start=(kp == 0),
                        stop=(kp == NKX // 2 - 1 and KT == NKX),
                        perf_mode=DR)
                if KT > NKX:
                    # bias tail tile: plain (non-DoubleRow) fp8 matmul
                    nc.tensor.matmul(pdt[:], W4[:, m, NKX:NKX + 1, :],
                                     x4[:, c, NKX:NKX + 1, :],
                                     start=False, stop=True)
                # ACT ops run async while PE streams the next k-loop
                nc.scalar.mul(dT_of[c][:, m * 512:m * 512 + 512],
                              pdt[:], DCOPY)
                sq = qp.tile([128, 512], f32, tag="sq", name=f"sq_{c}_{m}")
                nc.scalar.square(sq[:], pdt[:])
                sq_of[(c, m)] = sq

            def emit_leftover(c, m):
                # mean_base: per-(m,sub) single-shot ones-matmuls
                sq = sq_of.pop((c, m))
                mbp = mb_of[c]
                for sub in range(NSUB):
                    col = m * NSUB + sub
                    nc.tensor.matmul(mbp[:, col:col + 1],
                                     sq[:, sub * 128:sub * 128 + 128],
                                     onesH[:], start=True, stop=True)
                if m % 2 == 1:
                    emit_pmm(c, m // 2, (0, 2))

            def emit_oddsubs(c):
                # subs 1/3 full chains after subs 0/2 committed (stop at mp7)
                emit_u(c, (1, 3))
                for mp in range(8):
                    emit_pmm(c, mp, (1, 3))

            def emit_consume(c):
                Vt = V_of.pop(c)
                dT_of.pop(c)
                # mean_base: reduce the 64 per-(m,sub) partials over m
                mbp = mb_of.pop(c)
                nc.vector.tensor_reduce(
                    mb_sb[:, c * NSUB:(c + 1) * NSUB],
                    mbp[:].rearrange("p (m s) -> p s m", m=NM),
                    axis=AX.X, op=ALU.add)
                for sub in range(NSUB):
                    ch = c * NSUB + sub
                    ab = acat_sb[:, ch * 32:(ch + 1) * 32].rearrange(
                        "p (t r) -> p t r", t=2)
                    ab = ab.unsqueeze(2).broadcast_to([128, 2, 8, 16])
                    va = Vt[sub // 2][:, (sub % 2) * 256:(sub % 2) * 256 + 256]
                    prod = vp.tile([128, 256], f32, tag="prod",
                                   name=f"prod_{ch}")
                    nc.vector.tensor_tensor(
                        prod[:].rearrange("p (t e r) -> p t e r", t=2, e=8),
                        va.rearrange("p (t e r) -> p t e r", t=2, e=8),
                        ab, ALU.mult)
                    red = vp.tile([128, 16], f32, tag="red", name=f"red_{ch}")
                    nc.vector.tensor_reduce(
                        red[:],
                        prod[:].rearrange("p (t e r) -> p t e r", t=2, e=8),
                        axis=AX.X, op=ALU.add)
                    nc.vector.tensor_add(mse_sb[:, ch * 8:(ch + 1) * 8],
                                         red[:, 0:8], red[:, 8:16])

            # ---- main loop: PE consumers of ACT outputs deferred one m ----
            pending = []
            for c in range(NC4):
                emit_start(c)
                for m in range(NM):
                    emit_kloop(c, m)
                    if pending:
                        pc, pm_ = pending.pop(0)
                        emit_leftover(pc, pm_)
                        if pm_ == NM - 1:
                            emit_oddsubs(pc)
                            emit_consume(pc)
                    pending.append((c, m))
            pc, pm_ = pending.pop(0)
            emit_leftover(pc, pm_)
            emit_oddsubs(pc)
            emit_consume(pc)

            # ---- feat partial ----
            scr1 = cp.tile([128, 128], f32, tag="scr1")
            fx = cp.tile([128, 1], f32, tag="fx")
            nc.vector.tensor_mul(scr1[:], mse_sb[:], wsel[:])
            nc.vector.tensor_reduce(fx[:], scr1[:], axis=AX.X, op=ALU.add)
            scr2 = cp.tile([128, 16], f32, tag="scr2")
            fmb = cp.tile([128, 1], f32, tag="fmb")
            nc.gpsimd.tensor_mul(scr2[:], mb_sb[:], wsele[:])
            nc.vector.tensor_reduce(fmb[:], scr2[:], axis=AX.X, op=ALU.add)
            fsum = cp.tile([128, 1], f32, tag="fsum")
            nc.vector.tensor_add(fsum[:], fx[:], fmb[:])
            fpt = pd.tile([128, 512], f32, tag="pd", name="fp")
            nc.tensor.matmul(fpt[0:1, 0:1], fsum[:], ones1[:],
                             start=True, stop=True)
            fout = cp.tile([1, 1], f32, tag="fout")
            nc.scalar.copy(fout[:], fpt[0:1, 0:1])
            nc.sync.dma_start(d_feat, fout[:])
            if debug_out:
                nc.sync.dma_start(d_msed, mse_sb[:])
                nc.sync.dma_start(d_mbd, mb_sb[:])

    nc.compile()
    return nc


def _get_program(db_nonzero: bool, debug_out: bool = False):
    key = (bool(db_nonzero), bool(debug_out))
    if key not in _PROGRAM_CACHE:
        _PROGRAM_CACHE[key] = _build_program(*key)
    return _PROGRAM_CACHE[key]


# ----------------------------------------------------------------------------
# host side
# ----------------------------------------------------------------------------

def _host_scan_all(tg_all, sg_all, mask_f, gumbel):
    """Method-A sampling scan, all cores vectorized. Exact argmax semantics.
    Returns (wsel[B,S,E] f32, wsum f64, t_counts[E] f64, s_counts[E] f64)."""
    f32 = np.float32
    p = tg_all.astype(f32).copy()
    wsel = np.zeros((B, S, E), f32)
    BIG = f32(1e4)
    iota = np.arange(E, dtype=f32)
    for k in range(K):
        z = np.log(p) + gumbel[k]
        m = z.max(-1, keepdims=True)
        ge = (z >= m).astype(f32)
        t = iota + BIG - BIG * ge
        idxf = t.min(-1, keepdims=True)
        oh = (iota == idxf).astype(f32)
        po = p * oh
        w = po.sum(-1)
        mw = mask_f * w
        wsel += mw[..., None] * oh
        if k < K - 1:
            pn = p + (ALPHA - 1.0) * po
            p = pn / pn.sum(-1, keepdims=True)
    t_counts = wsel.astype(np.float64).sum(axis=(0, 1))
    wsum = float(t_counts.sum())
    # recompute s-side accumulation (needs per-step oh); cheap second pass
    p = tg_all.astype(f32).copy()
    s_counts = np.zeros(E, np.float64)
    for k in range(K):
        z = np.log(p) + gumbel[k]
        m = z.max(-1, keepdims=True)
        ge = (z >= m).astype(f32)
        t = iota + BIG - BIG * ge
        idxf = t.min(-1, keepdims=True)
        oh = (iota == idxf).astype(f32)
        po = p * oh
        sg_k = (sg_all * oh).sum(-1)
        s_counts += ((mask_f * sg_k)[..., None] * oh).astype(np.float64).sum(axis=(0, 1))
        if k < K - 1:
            pn = p + (ALPHA - 1.0) * po
            p = pn / pn.sum(-1, keepdims=True)
    return wsel, wsum, t_counts, s_counts


def _host_method_b(tg, sg, temp_c):
    """Per-core method-B partials: (tkl, ent)."""
    f32 = np.float32
    tg = tg.astype(f32)
    sg = sg.astype(f32)
    sgT = sg / f32(temp_c)
    ltg = np.log(tg)
    lsg = np.log(sg)
    ent = (sg * lsg).sum(dtype=f32)
    mb2 = sgT.max(-1, keepdims=True)
    ex = np.exp(sgT - mb2)
    se = ex.sum(-1, keepdims=True, dtype=f32)
    lse = np.log(se) + mb2
    sum_tg = tg.sum(-1, keepdims=True, dtype=f32)
    tkl = (tg * (ltg - sgT)).sum(dtype=f32) + (lse * sum_tg).sum(dtype=f32)
    return tkl, ent


def _prep_shared(inputs, db_nonzero):
    """Replicated (per-core identical) device arrays."""
    f32 = np.float32
    W_t = np.asarray(inputs["W_t"], f32)
    W_s = np.asarray(inputs["W_s"], f32)
    A_t = np.asarray(inputs["A_t"], f32)
    A_s = np.asarray(inputs["A_s"], f32)
    B_t = np.asarray(inputs["B_t"], f32)
    B_s = np.asarray(inputs["B_s"], f32)
    db = (np.asarray(inputs["b_s"], f32) - np.asarray(inputs["b_t"], f32))

    KT = NKX + (1 if db_nonzero else 0)

    # Wc layout [p, m, k*128 + c] = Wcat[m*128+c, k*128+p], fp8 pre-scaled.
    # k<16: W_s tiles; 16<=k<32: -W_t tiles; k=32 (db path): bias row on p=0.
    def w_tiles(W):
        return ((W * WSCALE).astype(FP8)
                .reshape(NM, 128, NM, 128).transpose(3, 0, 2, 1))  # [p,m,k,c]

    Wc = np.zeros((128, NM, KT, 128), FP8)
    Wc[:, :, 0:16, :] = w_tiles(W_s)
    Wc[:, :, 16:32, :] = w_tiles(-W_t)
    if db_nonzero:
        Wc[0, :, 32, :] = (db * WSCALE).astype(FP8).reshape(NM, 128)
    Wc = np.ascontiguousarray(Wc.reshape(128, NM, KT * 128))

    # Bc [p, mp, j, 256]: j = DoubleRow pair index (m = 2mp+j); 256 cols =
    # [BC_F*B_s_her | -BC_F*B_t_her] for that m's h-block.
    Bs_her = B_s.transpose(1, 0, 2).reshape(H, E * R)
    Bt_her = B_t.transpose(1, 0, 2).reshape(H, E * R)
    Bfull = np.concatenate([BC_F * Bs_her, -BC_F * Bt_her], axis=1)  # [H,256]
    Bc = np.ascontiguousarray(
        Bfull.reshape(8, 2, 128, 256).transpose(2, 0, 1, 3)
        .reshape(128, 8 * 512)).astype(FP8)

    # Gram pair strips [16, 2, 256] fp8, sharing the V accumulator's ALPHA_V
    G_ss = np.einsum("ehr,ehq->erq", B_s, B_s)
    G_st = np.einsum("ehr,ehq->erq", B_s, B_t)
    G_tt = np.einsum("ehr,ehq->erq", B_t, B_t)
    G_stT = G_st.transpose(0, 2, 1)

    def to_req(G):
        return G.transpose(1, 0, 2).reshape(R, E * R)

    Gs = np.concatenate([GC_F * to_req(G_ss), -GC_F * to_req(G_st)], axis=1)
    Gt = np.concatenate([-GC_F * to_req(G_stT), GC_F * to_req(G_tt)], axis=1)
    Gc = np.ascontiguousarray(
        np.stack([Gs, Gt], axis=1).reshape(16, 512)).astype(FP8)

    onesH = np.full((128, 1), 1.0 / (H * WSCALE * WSCALE), f32)
    ones1 = np.ones((128, 1), f32)

    shared = dict(Wc=Wc, Bc=Bc, Gc=Gc, onesH=onesH, ones1=ones1)
    mats = dict(A_sT=np.ascontiguousarray(A_s.T), A_tT=np.ascontiguousarray(A_t.T))
    return shared, mats, KT


def _prep_core(inputs, core, KT, wsel, mats):
    """Per-core device arrays."""
    f32 = np.float32
    sh = np.asarray(inputs["student_hidden_states"][core], f32)
    th = np.asarray(inputs["teacher_hidden_states"][core], f32)

    a_s = sh @ mats["A_sT"]                      # [S, R] f32
    a_t = th @ mats["A_tT"]
    acat = np.concatenate([a_s, a_t], axis=1) * f32(1.0 / ALPHA_V)  # [S, 32]
    acat = np.ascontiguousarray(
        acat.reshape(NCH, 128, 32).transpose(1, 0, 2).reshape(128, NCH * 32)
    ).astype(f32)
    aT = np.ascontiguousarray(
        np.stack([a_s.T, a_t.T], axis=1).reshape(16, 2 * S)).astype(FP8)

    # xc [p, chunk, k, s]: k<16 student, 16<=k<32 teacher, k=32 ones (db path)
    def x_tiles(x):
        return (x.T.astype(FP8).reshape(16, 128, NC4, 512)
                .transpose(1, 2, 0, 3))          # [p, c, k, s]

    xcv = np.zeros((128, NC4, KT, 512), FP8)
    xcv[:, :, 0:16, :] = x_tiles(sh)
    xcv[:, :, 16:32, :] = x_tiles(th)
    if KT > NKX:
        xcv[0, :, 32, :] = FP8(1.0)
    xcv = np.ascontiguousarray(xcv.reshape(128, NC4, KT * 512))

    wsel_dev = np.ascontiguousarray(
        wsel.reshape(NCH, 128, E).transpose(1, 0, 2).reshape(128, 128)).astype(f32)
    wsel_e = np.ascontiguousarray(wsel.sum(-1).reshape(NCH, 128).T).astype(f32)
    return dict(xc=xcv, wsel=wsel_dev, wsel_e=wsel_e, acat=acat, aT=aT)


def _combine(feat_parts, wsum, t_counts, s_counts, tkls, ents, temp_c):
    f32 = np.float32
    feat = np.sum(np.asarray(feat_parts, f32), dtype=f32)
    tc = np.asarray(t_counts, np.float64)
    sc = np.asarray(s_counts, np.float64)
    tkl = np.sum(np.asarray(tkls, f32), dtype=f32)
    ent = np.sum(np.asarray(ents, f32), dtype=f32)

    feat_loss = feat / max(wsum, 1e-8)
    t_avg = tc / tc.sum() + EPS
    s_avg = sc / sc.sum() + EPS
    t_avg = t_avg / t_avg.sum()
    s_avg = s_avg / s_avg.sum()
    coverage_kl = (t_avg * (np.log(t_avg) - np.log(s_avg))).sum() / E
    method_a_total = feat_loss + LAMBDA_COV * coverage_kl
    temp_kl = tkl / B
    entropy_loss = ent / (B * S)
    method_b_total = temp_kl + BETA_ENT * entropy_loss
    return np.array(
        [feat_loss, coverage_kl, method_a_total, temp_kl, entropy_loss,
         method_b_total, temp_c], f32)


def _host_all(inputs):
    """Host scan/method-B for all cores + per-core device input maps."""
    f32 = np.float32
    db_nonzero = bool(
        np.any(np.asarray(inputs["b_s"], f32) != np.asarray(inputs["b_t"], f32)))
    temp = float(np.asarray(inputs["temperature"], f32))
    temp_c = float(np.clip(temp, TEMP_LO, TEMP_HI))

    u = np.asarray(inputs["uniform_noise"], f32)
    gumbel = -np.log(-np.log(u * (1.0 - 2e-7) + 1e-7)).astype(f32)
    mask_f = np.asarray(inputs["attention_mask"], f32)
    tg_all = np.asarray(inputs["teacher_gates"], f32)
    sg_all = np.asarray(inputs["student_gates"], f32)

    shared, mats, KT = _prep_shared(inputs, db_nonzero)
    wsel_all, wsum, t_counts, s_counts = _host_scan_all(
        tg_all, sg_all, mask_f, gumbel)

    in_maps = []
    tkls, ents = [], []
    for c in range(B):
        tkl, ent = _host_method_b(tg_all[c], sg_all[c], temp_c)
        tkls.append(tkl)
        ents.append(ent)
        m = dict(shared)
        m.update(_prep_core(inputs, c, KT, wsel_all[c], mats))
        in_maps.append(m)

    return dict(in_maps=in_maps, db_nonzero=db_nonzero, temp_c=temp_c,
                wsum=wsum, t_counts=t_counts, s_counts=s_counts,
                tkls=tkls, ents=ents)


def kernel(**inputs) -> np.ndarray:
    host = _host_all(inputs)
    nc = _get_program(host["db_nonzero"])

    from concourse.bass_utils import run_bass_kernel_spmd

    res = run_bass_kernel_spmd(nc, host["in_maps"], core_ids=list(range(B)))
    feat_parts = [float(res.results[c]["feat"][0, 0]) for c in range(B)]

    return _combine(feat_parts, host["wsum"], host["t_counts"],
                    host["s_counts"], host["tkls"], host["ents"],
                    host["temp_c"])


# revision 18
# speedup vs baseline: 1.3726x; 1.0648x over previous
"""Trainium2 Bass kernel for nn_ExpertDistillationLoss.

Strategy (data-parallel over batch, 8 cores, 1 batch element each):
  - Device (per core): the FLOP-heavy expert-MSE pipeline.
      d.T[h, s] = W_s.sh.T - W_t.th.T computed as one concatenated fp8
      DoubleRow GEMM (W stationary & SBUF-resident, loaded once; host
      pre-transposed layouts; f32 PSUM accumulation).
      mean_base via ACT square + per-tile ones-matmul PSUM accumulation.
      cross+quad terms fused into one PSUM accumulator V[s, 256] built from
      (a) fp8 DoubleRow P-matmuls of dT m-tile PAIRS against host-prescaled
          B_cat and
      (b) one fp8 DoubleRow Gram matmul per token tile (as/at paired),
      then a broadcasted DVE multiply/reduce against a_s/a_t.
      Device output per core: feat partial = sum wsel*mse (1 scalar).
  - Host: input sharding/layout, the K=3 MC sampling scan (gates-only, exact
    argmax semantics), method-B losses, and the final scalar combine.
"""

import numpy as np
import ml_dtypes

B, S, H, E, R, K = 8, 2048, 2048, 8, 16, 3
ALPHA = 0.5
LAMBDA_COV = 0.5
BETA_ENT = 0.1
TEMP_LO, TEMP_HI = 0.5, 1.5
SCALE_T = 2.0
SCALE_S = 2.0
EPS = 1e-8

NM = 16                # output h-tiles (128 rows each)
NKX = 32               # k-tiles: 16 student + 16 teacher
NC4 = 4                # 512-token chunks
NSUB = 4               # 128-token subchunks per chunk
NCH = 16               # 128-token chunks over S

BF16 = ml_dtypes.bfloat16
FP8 = ml_dtypes.float8_e4m3fn
WSCALE = 64.0          # W pre-scale so fp8 e4m3 stays in normal range
DCOPY = 0.25           # dT = DCOPY * pd = (WSCALE*DCOPY) * d = 16 d
ALPHA_V = 131072.0     # 2**17: common scale carried by the V accumulator
BC_F = ALPHA_V * 2.0 * SCALE_S / (H * WSCALE * DCOPY)   # = 16.0
GC_F = ALPHA_V * SCALE_S * SCALE_T / H                  # = 256.0

_PROGRAM_CACHE = {}


# ----------------------------------------------------------------------------
# device program
# ----------------------------------------------------------------------------

def _build_program(db_nonzero: bool, debug_out: bool = False):
    import concourse.bacc as bacc
    import concourse.tile as tile
    from concourse import mybir

    f32 = mybir.dt.float32
    fp8 = mybir.dt.float8e4
    DR = mybir.MatmulPerfMode.DoubleRow
    ALU = mybir.AluOpType
    AX = mybir.AxisListType

    KT = NKX + (1 if db_nonzero else 0)   # extra k-tile carries the bias row
    WB = KT * 128                          # W cols per m-tile
    XB = KT * 512                          # x cols per 512-token chunk

    nc = bacc.Bacc("TRN2", target_bir_lowering=False, debug=False)

    # DRAM inputs (per-core shapes; layouts are host-prepared)
    d_xc = nc.dram_tensor("xc", [128, NC4, XB], fp8, kind="ExternalInput").ap()
    d_Wc = nc.dram_tensor("Wc", [128, NM, WB], fp8, kind="ExternalInput").ap()
    d_Bc = nc.dram_tensor("Bc", [128, 8 * 512], fp8, kind="ExternalInput").ap()
    d_Gc = nc.dram_tensor("Gc", [16, 512], fp8, kind="ExternalInput").ap()
    d_aT = nc.dram_tensor("aT", [16, 2 * S], fp8, kind="ExternalInput").ap()
    d_ac = nc.dram_tensor("acat", [128, NCH * 32], f32, kind="ExternalInput").ap()
    d_wsel = nc.dram_tensor("wsel", [128, 128], f32, kind="ExternalInput").ap()
    d_wsele = nc.dram_tensor("wsel_e", [128, 16], f32, kind="ExternalInput").ap()
    d_onesH = nc.dram_tensor("onesH", [128, 1], f32, kind="ExternalInput").ap()

    d_feat = nc.dram_tensor("feat", [128, 1], f32, kind="ExternalOutput").ap()
    if debug_out:
        d_msed = nc.dram_tensor("mse_dbg", [128, 128], f32, kind="ExternalOutput").ap()
        d_mbd = nc.dram_tensor("mb_dbg", [128, 16], f32, kind="ExternalOutput").ap()

    with tile.TileContext(nc) as tc:
        with (
            tc.tile_pool(name="const", bufs=1) as cp,
            tc.tile_pool(name="dT", bufs=2) as dp,
            tc.tile_pool(name="sq", bufs=3) as qp,
            tc.tile_pool(name="vc", bufs=4) as vp,
            tc.tile_pool(name="pd", bufs=2, space="PSUM") as pd,
            tc.tile_pool(name="pv", bufs=4, space="PSUM") as pv,
            tc.tile_pool(name="pm", bufs=2, space="PSUM") as pm,
        ):
            # ---- SBUF tiles ----
            Gc_sb = cp.tile([16, 512], fp8, tag="Gc")
            aT_sb = cp.tile([16, 2 * S], fp8, tag="aT")
            Wc = cp.tile([128, NM * WB], fp8, tag="Wc")
            xc = cp.tile([128, NC4 * XB], fp8, tag="xc")
            Bc = cp.tile([128, 8 * 512], fp8, tag="Bc")
            acat_sb = cp.tile([128, NCH * 32], f32, tag="acat")
            wsel = cp.tile([128, 128], f32, tag="wsel")
            wsele = cp.tile([128, 16], f32, tag="wsele")
            onesH = cp.tile([128, 1], f32, tag="onesH")
            mse_sb = cp.tile([128, 128], f32, tag="mse")
            mb_sb = cp.tile([128, 16], f32, tag="mb")
            facc = cp.tile([128, 1], f32, tag="facc")
            nc.vector.memset(facc[:], 0.0)

            # ---- DMA emission order (HWDGE serializes at ~625ns/DMA and the
            # DMA bus at ~360B/ns shared; order = need order on the PE).
            # W must stream ahead of the PE's ~1.9us/m-tile cadence, so after
            # chunk 0's x data the W tiles go out back-to-back; later x chunks
            # ride behind the full W set.
            dma = nc.sync.dma_start
            dma(Gc_sb[:], d_Gc)
            dma(aT_sb[:], d_aT)
            dma(Wc[:, 0:256], d_Wc[:, 0, 0:256])          # m0 kp0
            dma(xc[:, 0:1024], d_xc[:, 0, 0:1024])        # c0 kp0
            dma(Wc[:, 256:WB], d_Wc[:, 0, 256:WB])        # m0 rest
            dma(xc[:, 1024:4096], d_xc[:, 0, 1024:4096])  # c0 kp1-3
            dma(xc[:, 4096:8192], d_xc[:, 0, 4096:8192])  # c0 kp4-7
            dma(xc[:, 8192:12288], d_xc[:, 0, 8192:12288])
            dma(xc[:, 12288:XB], d_xc[:, 0, 12288:XB])
            dma(Wc[:, WB:2 * WB], d_Wc[:, 1, :])
            dma(Bc[:], d_Bc)
            dma(onesH[:], d_onesH)
            for m in range(2, 16):
                dma(Wc[:, m * WB:(m + 1) * WB], d_Wc[:, m, :])
            for q in range(4):                             # c1 in 4 pieces
                dma(xc[:, XB + q * 4096:XB + (q + 1) * 4096],
                    d_xc[:, 1, q * 4096:(q + 1) * 4096])
            dma(acat_sb[:], d_ac)
            dma(wsel[:], d_wsel)
            dma(wsele[:], d_wsele)
            dma(xc[:, 2 * XB:3 * XB], d_xc[:, 2, :])
            dma(xc[:, 3 * XB:4 * XB], d_xc[:, 3, :])

            # ---- views ----
            W4 = Wc[:].rearrange("p (m k c) -> p m k c", m=NM, k=KT)
            x4 = xc[:].rearrange("p (n k s) -> p n k s", n=NC4, k=KT)
            aT2 = aT_sb[:].rearrange("p (j s) -> p j s", j=2)
            Gc2 = Gc_sb[:].rearrange("p (j g) -> p j g", j=2)
            Bc3 = Bc[:].rearrange("p (mp j g) -> p mp j g", mp=8, j=2)

            # PSUM rule (probe-verified): a bank holds ONE open accumulation
            # group; a start=True wipes any other OPEN group's partials in
            # that bank (committed/stopped results survive). So: V banks run
            # one sub's full chain at a time (subs 0/2 during the m-loop,
            # subs 1/3 afterwards from the dT cache), and mean_base uses
            # per-(m,sub) single-shot matmuls + a DVE reduction over m.
            V_of = {}    # c -> [2 psum tiles of [128, 512] (2 subs each)]
            mb_of = {}   # c -> [128, 64] psum tile of per-(m,sub) sums
            sq_of = {}   # (c, m) -> sq tile
            dT_of = {}   # c -> [128, 8*1024] fp8 dT cache (mp, j, 512)

            def Vap(c, sub):
                t = V_of[c][sub // 2]
                return t[:, (sub % 2) * 256:(sub % 2) * 256 + 256]

            def emit_u(c, subs):
                for sub in subs:
                    t0 = c * 512 + sub * 128
                    nc.tensor.matmul(Vap(c, sub), aT2[:, :, t0:t0 + 128],
                                     Gc2, start=True, stop=False,
                                     perf_mode=DR)

            def emit_pmm(c, mp, subs):
                dT3 = dT_of[c][:].rearrange("p (mp j s) -> p mp j s",
                                            mp=8, j=2)
                for sub in subs:
                    nc.tensor.matmul(
                        Vap(c, sub),
                        dT3[:, mp, :, sub * 128:sub * 128 + 128],
                        Bc3[:, mp], start=False, stop=(mp == 7),
                        perf_mode=DR)

            def emit_start(c):
                V_of[c] = [pv.tile([128, 512], f32, tag="V", name=f"V_{c}_{i}")
                           for i in range(2)]
                mb_of[c] = pm.tile([128, 64], f32, tag="mbp", name=f"mb_{c}")
                dT_of[c] = dp.tile([128, 8 * 1024], fp8, tag="dT",
                                   name=f"dTall_{c}")
                emit_u(c, (0, 2))

            def emit_kloop(c, m):
                pdt = pd.tile([128, 512], f32, tag="pd", name=f"pd_{c}_{m}")
                for kp in range(NKX // 2):
                    nc.tensor.matmul(
                        pdt[:], W4[:, m, 2 * kp:2 * kp + 2, :],
                        x4[:, c, 2 * kp:2 * kp + 2, :],
                        start=(kp == 0),
                        stop=(kp == NKX // 2 - 1 and KT == NKX),
                        perf_mode=DR)
                if KT > NKX:
                    # bias tail tile: plain (non-DoubleRow) fp8 matmul
                    nc.tensor.matmul(pdt[:], W4[:, m, NKX:NKX + 1, :],
                                     x4[:, c, NKX:NKX + 1, :],
                                     start=False, stop=True)
                # ACT ops run async while PE streams the next k-loop
                nc.scalar.mul(dT_of[c][:, m * 512:m * 512 + 512],
                              pdt[:], DCOPY)
                sq = qp.tile([128, 512], f32, tag="sq", name=f"sq_{c}_{m}")
                nc.scalar.square(sq[:], pdt[:])
                sq_of[(c, m)] = sq

            def emit_leftover(c, m):
                # mean_base: per-(m,sub) single-shot ones-matmuls
                sq = sq_of.pop((c, m))
                mbp = mb_of[c]
                for sub in range(NSUB):
                    col = m * NSUB + sub
                    nc.tensor.matmul(mbp[:, col:col + 1],
                                     sq[:, sub * 128:sub * 128 + 128],
                                     onesH[:], start=True, stop=True)
                if m % 2 == 1:
                    emit_pmm(c, m // 2, (0, 2))

            def emit_oddsubs(c):
                # subs 1/3 full chains after subs 0/2 committed (stop at mp7)
                emit_u(c, (1, 3))
                for mp in range(8):
                    emit_pmm(c, mp, (1, 3))

            def emit_consume(c, subs):
                for sub in subs:
                    ch = c * NSUB + sub
                    ab = acat_sb[:, ch * 32:(ch + 1) * 32].rearrange(
                        "p (t r) -> p t r", t=2)
                    ab = ab.unsqueeze(2).broadcast_to([128, 2, 8, 16])
                    va = Vap(c, sub)
                    prod = vp.tile([128, 256], f32, tag="prod",
                                   name=f"prod_{ch}")
                    nc.vector.tensor_tensor(
                        prod[:].rearrange("p (t e r) -> p t e r", t=2, e=8),
                        va.rearrange("p (t e r) -> p t e r", t=2, e=8),
                        ab, ALU.mult)
                    red = vp.tile([128, 16], f32, tag="red", name=f"red_{ch}")
                    nc.vector.tensor_reduce(
                        red[:],
                        prod[:].rearrange("p (t e r) -> p t e r", t=2, e=8),
                        axis=AX.X, op=ALU.add)
                    nc.vector.tensor_add(mse_sb[:, ch * 8:(ch + 1) * 8],
                                         red[:, 0:8], red[:, 8:16])

            def emit_feat(c):
                # fold this chunk's mse and mean_base into the running facc
                V_of.pop(c)
                dT_of.pop(c)
                mbp = mb_of.pop(c)
                nc.vector.tensor_reduce(
                    mb_sb[:, c * NSUB:(c + 1) * NSUB],
                    mbp[:].rearrange("p (m s) -> p s m", m=NM),
                    axis=AX.X, op=ALU.add)
                scr = vp.tile([128, 32], f32, tag="scr", name=f"scr_{c}")
                nc.vector.tensor_mul(scr[:], mse_sb[:, c * 32:(c + 1) * 32],
                                     wsel[:, c * 32:(c + 1) * 32])
                red = vp.tile([128, 1], f32, tag="fred", name=f"fred_{c}")
                nc.vector.tensor_reduce(red[:], scr[:], axis=AX.X, op=ALU.add)
                nc.vector.tensor_add(facc[:], facc[:], red[:])
                scr2 = vp.tile([128, 4], f32, tag="scr2", name=f"scr2_{c}")
                nc.vector.tensor_mul(scr2[:], mb_sb[:, c * 4:(c + 1) * 4],
                                     wsele[:, c * 4:(c + 1) * 4])
                red2 = vp.tile([128, 1], f32, tag="fred2", name=f"fred2_{c}")
                nc.vector.tensor_reduce(red2[:], scr2[:], axis=AX.X,
                                        op=ALU.add)
                nc.vector.tensor_add(facc[:], facc[:], red2[:])

            # ---- main loop: PE consumers of ACT outputs deferred one m ----
            pending = []
            for c in range(NC4):
                emit_start(c)
                for m in range(NM):
                    emit_kloop(c, m)
                    if pending:
                        pc, pm_ = pending.pop(0)
                        emit_leftover(pc, pm_)
                        if pm_ == NM - 1:
                            emit_oddsubs(pc)
                            emit_consume(pc, (0, 1, 2, 3))
                            emit_feat(pc)
                    pending.append((c, m))
            pc, pm_ = pending.pop(0)
            emit_leftover(pc, pm_)
            emit_consume(pc, (0, 2))   # subs 0/2 committed; DVE runs while
            emit_oddsubs(pc)           # ...PE finishes subs 1/3
            emit_consume(pc, (1, 3))
            emit_feat(pc)
            nc.sync.dma_start(d_feat, facc[:])
            if debug_out:
                nc.sync.dma_start(d_msed, mse_sb[:])
                nc.sync.dma_start(d_mbd, mb_sb[:])

    nc.compile()
    return nc


def _get_program(db_nonzero: bool, debug_out: bool = False):
    key = (bool(db_nonzero), bool(debug_out))
    if key not in _PROGRAM_CACHE:
        _PROGRAM_CACHE[key] = _build_program(*key)
    return _PROGRAM_CACHE[key]


# ----------------------------------------------------------------------------
# host side
# ----------------------------------------------------------------------------

def _host_scan_all(tg_all, sg_all, mask_f, gumbel):
    """Method-A sampling scan, all cores vectorized. Exact argmax semantics.
    Returns (wsel[B,S,E] f32, wsum f64, t_counts[E] f64, s_counts[E] f64)."""
    f32 = np.float32
    p = tg_all.astype(f32).copy()
    wsel = np.zeros((B, S, E), f32)
    BIG = f32(1e4)
    iota = np.arange(E, dtype=f32)
    for k in range(K):
        z = np.log(p) + gumbel[k]
        m = z.max(-1, keepdims=True)
        ge = (z >= m).astype(f32)
        t = iota + BIG - BIG * ge
        idxf = t.min(-1, keepdims=True)
        oh = (iota == idxf).astype(f32)
        po = p * oh
        w = po.sum(-1)
        mw = mask_f * w
        wsel += mw[..., None] * oh
        if k < K - 1:
            pn = p + (ALPHA - 1.0) * po
            p = pn / pn.sum(-1, keepdims=True)
    t_counts = wsel.astype(np.float64).sum(axis=(0, 1))
    wsum = float(t_counts.sum())
    # recompute s-side accumulation (needs per-step oh); cheap second pass
    p = tg_all.astype(f32).copy()
    s_counts = np.zeros(E, np.float64)
    for k in range(K):
        z = np.log(p) + gumbel[k]
        m = z.max(-1, keepdims=True)
        ge = (z >= m).astype(f32)
        t = iota + BIG - BIG * ge
        idxf = t.min(-1, keepdims=True)
        oh = (iota == idxf).astype(f32)
        po = p * oh
        sg_k = (sg_all * oh).sum(-1)
        s_counts += ((mask_f * sg_k)[..., None] * oh).astype(np.float64).sum(axis=(0, 1))
        if k < K - 1:
            pn = p + (ALPHA - 1.0) * po
            p = pn / pn.sum(-1, keepdims=True)
    return wsel, wsum, t_counts, s_counts


def _host_method_b(tg, sg, temp_c):
    """Per-core method-B partials: (tkl, ent)."""
    f32 = np.float32
    tg = tg.astype(f32)
    sg = sg.astype(f32)
    sgT = sg / f32(temp_c)
    ltg = np.log(tg)
    lsg = np.log(sg)
    ent = (sg * lsg).sum(dtype=f32)
    mb2 = sgT.max(-1, keepdims=True)
    ex = np.exp(sgT - mb2)
    se = ex.sum(-1, keepdims=True, dtype=f32)
    lse = np.log(se) + mb2
    sum_tg = tg.sum(-1, keepdims=True, dtype=f32)
    tkl = (tg * (ltg - sgT)).sum(dtype=f32) + (lse * sum_tg).sum(dtype=f32)
    return tkl, ent


def _prep_shared(inputs, db_nonzero):
    """Replicated (per-core identical) device arrays."""
    f32 = np.float32
    W_t = np.asarray(inputs["W_t"], f32)
    W_s = np.asarray(inputs["W_s"], f32)
    A_t = np.asarray(inputs["A_t"], f32)
    A_s = np.asarray(inputs["A_s"], f32)
    B_t = np.asarray(inputs["B_t"], f32)
    B_s = np.asarray(inputs["B_s"], f32)
    db = (np.asarray(inputs["b_s"], f32) - np.asarray(inputs["b_t"], f32))

    KT = NKX + (1 if db_nonzero else 0)

    # Wc layout [p, m, k*128 + c] = Wcat[m*128+c, k*128+p], fp8 pre-scaled.
    # k<16: W_s tiles; 16<=k<32: -W_t tiles; k=32 (db path): bias row on p=0.
    def w_tiles(W):
        return ((W * WSCALE).astype(FP8)
                .reshape(NM, 128, NM, 128).transpose(3, 0, 2, 1))  # [p,m,k,c]

    Wc = np.zeros((128, NM, KT, 128), FP8)
    Wc[:, :, 0:16, :] = w_tiles(W_s)
    Wc[:, :, 16:32, :] = w_tiles(-W_t)
    if db_nonzero:
        Wc[0, :, 32, :] = (db * WSCALE).astype(FP8).reshape(NM, 128)
    Wc = np.ascontiguousarray(Wc.reshape(128, NM, KT * 128))

    # Bc [p, mp, j, 256]: j = DoubleRow pair index (m = 2mp+j); 256 cols =
    # [BC_F*B_s_her | -BC_F*B_t_her] for that m's h-block.
    Bs_her = B_s.transpose(1, 0, 2).reshape(H, E * R)
    Bt_her = B_t.transpose(1, 0, 2).reshape(H, E * R)
    Bfull = np.concatenate([BC_F * Bs_her, -BC_F * Bt_her], axis=1)  # [H,256]
    Bc = np.ascontiguousarray(
        Bfull.reshape(8, 2, 128, 256).transpose(2, 0, 1, 3)
        .reshape(128, 8 * 512)).astype(FP8)

    # Gram pair strips [16, 2, 256] fp8, sharing the V accumulator's ALPHA_V
    G_ss = np.einsum("ehr,ehq->erq", B_s, B_s)
    G_st = np.einsum("ehr,ehq->erq", B_s, B_t)
    G_tt = np.einsum("ehr,ehq->erq", B_t, B_t)
    G_stT = G_st.transpose(0, 2, 1)

    def to_req(G):
        return G.transpose(1, 0, 2).reshape(R, E * R)

    Gs = np.concatenate([GC_F * to_req(G_ss), -GC_F * to_req(G_st)], axis=1)
    Gt = np.concatenate([-GC_F * to_req(G_stT), GC_F * to_req(G_tt)], axis=1)
    Gc = np.ascontiguousarray(
        np.stack([Gs, Gt], axis=1).reshape(16, 512)).astype(FP8)

    onesH = np.full((128, 1), 1.0 / (H * WSCALE * WSCALE), f32)

    shared = dict(Wc=Wc, Bc=Bc, Gc=Gc, onesH=onesH)
    mats = dict(A_sT=np.ascontiguousarray(A_s.T), A_tT=np.ascontiguousarray(A_t.T))
    return shared, mats, KT


def _prep_core(inputs, core, KT, wsel, mats):
    """Per-core device arrays."""
    f32 = np.float32
    sh = np.asarray(inputs["student_hidden_states"][core], f32)
    th = np.asarray(inputs["teacher_hidden_states"][core], f32)

    a_s = sh @ mats["A_sT"]                      # [S, R] f32
    a_t = th @ mats["A_tT"]
    acat = np.concatenate([a_s, a_t], axis=1) * f32(1.0 / ALPHA_V)  # [S, 32]
    acat = np.ascontiguousarray(
        acat.reshape(NCH, 128, 32).transpose(1, 0, 2).reshape(128, NCH * 32)
    ).astype(f32)
    aT = np.ascontiguousarray(
        np.stack([a_s.T, a_t.T], axis=1).reshape(16, 2 * S)).astype(FP8)

    # xc [p, chunk, k, s]: k<16 student, 16<=k<32 teacher, k=32 ones (db path)
    def x_tiles(x):
        return (x.T.astype(FP8).reshape(16, 128, NC4, 512)
                .transpose(1, 2, 0, 3))          # [p, c, k, s]

    xcv = np.zeros((128, NC4, KT, 512), FP8)
    xcv[:, :, 0:16, :] = x_tiles(sh)
    xcv[:, :, 16:32, :] = x_tiles(th)
    if KT > NKX:
        xcv[0, :, 32, :] = FP8(1.0)
    xcv = np.ascontiguousarray(xcv.reshape(128, NC4, KT * 512))

    wsel_dev = np.ascontiguousarray(
        wsel.reshape(NCH, 128, E).transpose(1, 0, 2).reshape(128, 128)).astype(f32)
    wsel_e = np.ascontiguousarray(wsel.sum(-1).reshape(NCH, 128).T).astype(f32)
    return dict(xc=xcv, wsel=wsel_dev, wsel_e=wsel_e, acat=acat, aT=aT)


def _combine(feat_parts, wsum, t_counts, s_counts, tkls, ents, temp_c):
    f32 = np.float32
    feat = np.sum(np.asarray(feat_parts, f32), dtype=f32)
    tc = np.asarray(t_counts, np.float64)
    sc = np.asarray(s_counts, np.float64)
    tkl = np.sum(np.asarray(tkls, f32), dtype=f32)
    ent = np.sum(np.asarray(ents, f32), dtype=f32)

    feat_loss = feat / max(wsum, 1e-8)
    t_avg = tc / tc.sum() + EPS
    s_avg = sc / sc.sum() + EPS
    t_avg = t_avg / t_avg.sum()
    s_avg = s_avg / s_avg.sum()
    coverage_kl = (t_avg * (np.log(t_avg) - np.log(s_avg))).sum() / E
    method_a_total = feat_loss + LAMBDA_COV * coverage_kl
    temp_kl = tkl / B
    entropy_loss = ent / (B * S)
    method_b_total = temp_kl + BETA_ENT * entropy_loss
    return np.array(
        [feat_loss, coverage_kl, method_a_total, temp_kl, entropy_loss,
         method_b_total, temp_c], f32)


def _host_all(inputs):
    """Host scan/method-B for all cores + per-core device input maps."""
    f32 = np.float32
    db_nonzero = bool(
        np.any(np.asarray(inputs["b_s"], f32) != np.asarray(inputs["b_t"], f32)))
    temp = float(np.asarray(inputs["temperature"], f32))
    temp_c = float(np.clip(temp, TEMP_LO, TEMP_HI))

    u = np.asarray(inputs["uniform_noise"], f32)
    gumbel = -np.log(-np.log(u * (1.0 - 2e-7) + 1e-7)).astype(f32)
    mask_f = np.asarray(inputs["attention_mask"], f32)
    tg_all = np.asarray(inputs["teacher_gates"], f32)
    sg_all = np.asarray(inputs["student_gates"], f32)

    shared, mats, KT = _prep_shared(inputs, db_nonzero)
    wsel_all, wsum, t_counts, s_counts = _host_scan_all(
        tg_all, sg_all, mask_f, gumbel)

    in_maps = []
    tkls, ents = [], []
    for c in range(B):
        tkl, ent = _host_method_b(tg_all[c], sg_all[c], temp_c)
        tkls.append(tkl)
        ents.append(ent)
        m = dict(shared)
        m.update(_prep_core(inputs, c, KT, wsel_all[c], mats))
        in_maps.append(m)

    return dict(in_maps=in_maps, db_nonzero=db_nonzero, temp_c=temp_c,
                wsum=wsum, t_counts=t_counts, s_counts=s_counts,
                tkls=tkls, ents=ents)


def kernel(**inputs) -> np.ndarray:
    host = _host_all(inputs)
    nc = _get_program(host["db_nonzero"])

    from concourse.bass_utils import run_bass_kernel_spmd

    res = run_bass_kernel_spmd(nc, host["in_maps"], core_ids=list(range(B)))
    feat_parts = [float(np.asarray(res.results[c]["feat"], np.float32).sum())
                  for c in range(B)]

    return _combine(feat_parts, host["wsum"], host["t_counts"],
                    host["s_counts"], host["tkls"], host["ents"],
                    host["temp_c"])
